# revision 21
# baseline (speedup 1.0000x reference)
"""Trainium2 Bass kernel for nn_Attention_11836929868370.

8-core sharding: core c -> batch b = c//2, head group hg = c%2 (4 of 8 heads).
Each core computes its 4 heads' attention and a partial output projection;
the host sums the two partials per batch and adds the output bias.

Per-core pipeline (matmuls bf16, accumulation fp32 in PSUM):
  B.  Interleaved per 4-nt chunk:
        qkv = xT.T @ WqkvT (per-head LN mean subtraction folded into the
        weights host-side: W' = W - colmean(W)); ACT evacuates t and v from
        PSUM; GPSIMD computes ssq (sq + reduce); DVE ropes k (no LN scale -
        deferred to exp) and streams k/q transposes out on the sync/scalar
        DMA queues; after each chunk a small Newton rsqrt gives rs for the
        chunk and DVE ropes+scales q. Phase C can start once the first half
        of the q tiles exist.
  C.  per (head, 1024-wide q-block): S^T[nk,nq] = k'' @ q''.T on PE in
      [128,512] single-PSUM-bank tiles (6-deep ring so the PE runs ahead and
      LDWEIGHTS hide in the reorder window); softmax exp split between ACT
      (native Exp, per-partition scale = rs_k) and DVE (Schraudolph bit
      trick: f32 mad + f32->int16 convert writes the bf16 bit pattern of exp
      directly). PV with stationary [v_h | ones] gives o^T and the softmax
      denominator in row 64. Normalization lags one block: 1/den = Exp(-Ln)
      on ACT (same table as Exp, no reload), PE replicates it across
      partitions only after the next block's S matmuls are queued, DVE
      applies it.
  D.  out = oT.T @ WoT partial projection, staged to SBUF, DMA out. Host
      adds out_b (+ the v-bias contribution, which commutes through softmax
      averaging).
"""

import sys

if "/opt/trn_rl_repo" not in sys.path:
    sys.path.insert(0, "/opt/trn_rl_repo")

from contextlib import ExitStack

import ml_dtypes
import numpy as np

import concourse.bass as bass
import concourse.mybir as mybir
import concourse.tile as tile
from concourse.bass_utils import run_bass_kernel_spmd

BF16 = mybir.dt.bfloat16
F32 = mybir.dt.float32
I16 = mybir.dt.int16
I32 = mybir.dt.int32

DIM, NH, HD = 512, 8, 64
N = 2048
EPS = 1e-6
THETA = 10000.0
NT = N // 128          # 16 n-tiles
CT = DIM // 128        # 4 c-tiles
NHC = 4                # heads per core
QB = 2                 # q blocks of 1024
KT = NT                # key tiles
QBW = N // QB          # 1024
W3 = 768               # qkv weight columns per core (q 256 | k 256 | v 256)
CHUNK = 4              # nt per rsqrt chunk
RSQRT_MAGIC = float(0x5F3759DF)
# Schraudolph fast-exp constants for bf16 bit patterns via int16:
# bits(e^v) ~= round(v * 128/ln2 + (127*128 - 0.04367*128)); +0.5 assumes the
# f32->int16 convert truncates (if it rounds, the extra half-LSB is harmless).
EXPA = 184.66502308201368
EXPB = 16250.91
# which key-tiles' exp runs on DVE (rest on ACT)
DVE_KT = frozenset((2, 7, 12))


# ---------------------------------------------------------------------------
# sync-wait legalization: this walrus build rejects >1 sync wait per
# instruction ("Too many sync wait commands"), while Tile's sem assignment
# emits several. Excess waits are hoisted onto NoOps placed immediately
# before the instruction on the same engine, which preserves ordering.
# ---------------------------------------------------------------------------

def legalize_sync_waits(nc, max_waits=1):
    n = 0
    for fn in nc.m.functions:
        for bb in fn.blocks:
            new_insts = []
            for inst in bb.instructions:
                si = inst.sync_info
                if si is not None and si.on_wait and len(si.on_wait) > max_waits:
                    movable = [w for w in si.on_wait if w.wait_reg is None]
                    pinned = [w for w in si.on_wait if w.wait_reg is not None]
                    budget = max(max_waits - len(pinned), 0)
                    cut = len(movable) - budget
                    keep, excess = movable[cut:], movable[:cut]
                    for i in range(0, len(excess), max_waits):
                        nop = mybir.InstNoOp(
                            name=f"I-waitsplit-{n}",
                            engine=inst.engine,
                            text_hint="waitsplit",
                            sync_info=mybir.SyncInfo(
                                on_wait=excess[i : i + max_waits], on_update=[]
                            ),
                        )
                        n += 1
                        new_insts.append(nop)
                    si.on_wait = keep + pinned
                new_insts.append(inst)
            bb.instructions[:] = new_insts
    return n


# ---------------------------------------------------------------------------
# device program
# ---------------------------------------------------------------------------

def build_program(with_qkv_bias=False, with_ln_bias=False):
    nc = bass.Bass("TRN2", target_bir_lowering=False, debug=False, num_devices=8)

    # [128, CT, 2048]: x transposed (c on partitions) and cast to bf16
    xT_d = nc.dram_tensor("xT", [128, CT * N], BF16, kind="ExternalInput").ap()
    # [128, CT, 768]: wq(256 perm, mean-folded) | wk(256 perm, mean-folded) | wv(256)
    wq_d = nc.dram_tensor("wqkvT", [128, CT * W3], BF16, kind="ExternalInput").ap()
    wo_d = nc.dram_tensor("woT", [64, NHC * DIM], BF16, kind="ExternalInput").ap()
    # [128, NT, 256]: C2q | S2q | C2k | S2k  (gains, q-scale folded in)
    tab_d = nc.dram_tensor("tab", [128, NT * 256], BF16, kind="ExternalInput").ap()
    if with_qkv_bias:
        b_d = nc.dram_tensor("brow", [1, W3], BF16, kind="ExternalInput").ap()
    if with_ln_bias:
        tln_d = nc.dram_tensor("tln", [128, NT * 512], BF16, kind="ExternalInput").ap()
    out_d = nc.dram_tensor("outp", [N, DIM], F32, kind="ExternalOutput").ap()

    with tile.TileContext(nc) as tc, ExitStack() as ctx:
        consts = ctx.enter_context(tc.tile_pool(name="consts", bufs=1))
        pers = ctx.enter_context(tc.tile_pool(name="pers", bufs=1))
        stage = ctx.enter_context(tc.tile_pool(name="stage", bufs=4))
        small = ctx.enter_context(tc.tile_pool(name="small", bufs=4))
        exps = ctx.enter_context(tc.tile_pool(name="exps", bufs=52))
        fes = ctx.enter_context(tc.tile_pool(name="fes", bufs=3))
        ps = ctx.enter_context(tc.tile_pool(name="ps", bufs=6, space="PSUM"))

        # constants
        wq_sb = consts.tile([128, CT, W3], BF16)
        nc.sync.dma_start(wq_sb[:], wq_d.rearrange("p (t f) -> p t f", t=CT))
        xT_sb = consts.tile([128, CT, N], BF16)
        xT_dv = xT_d.rearrange("p (t f) -> p t f", t=CT)
        for ct in range(CT):
            nc.sync.dma_start(xT_sb[:, ct], xT_dv[:, ct])
        wo_sb = consts.tile([64, NHC, DIM], BF16)
        nc.sync.dma_start(wo_sb[:], wo_d.rearrange("p (t f) -> p t f", t=NHC))
        tab_sb = consts.tile([128, NT, 256], BF16)
        nc.sync.dma_start(tab_sb[:], tab_d.rearrange("p (t f) -> p t f", t=NT))
        # row 64 of this tile is the lhsT for the replicate matmul
        # (it must share its base partition with the PSUM denominator row)
        onesf_sb = consts.tile([65, 128], F32)
        nc.vector.memset(onesf_sb[:], 1.0)
        if with_qkv_bias:
            b_sb = consts.tile([1, W3], BF16)
            nc.sync.dma_start(b_sb[:], b_d)
            ones_sb = consts.tile([1, 128], BF16)
            nc.vector.memset(ones_sb[:], 1.0)
        if with_ln_bias:
            tln_sb = consts.tile([128, NT, 512], BF16)
            nc.sync.dma_start(tln_sb[:], tln_d.rearrange("p (t f) -> p t f", t=NT))

        # persistent intermediates
        qT = [pers.tile([128, N], BF16, name=f"qT{i}") for i in range(2)]
        kT = [pers.tile([128, N], BF16, name=f"kT{i}") for i in range(2)]
        oT = [pers.tile([64, N], BF16, name=f"oTh{i}") for i in range(NHC)]
        # v with a ones column per head: PV row 64 is the softmax denominator
        v_sb = pers.tile([128, KT, NHC, 65], BF16)
        t_all = pers.tile([128, NT, 8, HD], BF16)
        ssq_all = pers.tile([128, NT, 8], F32)
        rs_sb = pers.tile([128, NT, 8], F32)
        a_sb = pers.tile([128, NT, 8], F32)
        d_t = pers.tile([128, NT, 8], F32, name="rsq_d")

        nc.vector.memset(v_sb[:, :, :, 64], 1.0)

        COPY = mybir.ActivationFunctionType.Copy
        EXP = mybir.ActivationFunctionType.Exp
        LN_F = mybir.ActivationFunctionType.Ln

        def rope4(dst, t4, tcol, nt):
            """dst = rope of t4 (4 heads) using tab columns at tcol."""
            u = stage.tile([128, 4, HD], BF16, name="u")
            w = stage.tile([128, 4, HD], BF16, name="w")
            nc.vector.tensor_mul(
                u[:],
                t4,
                tab_sb[:, nt, tcol : tcol + 64].unsqueeze(1).to_broadcast((128, 4, HD)),
            )
            for half in (0, 1):
                d_out = slice(half * 32, half * 32 + 32)
                d_in = slice((1 - half) * 32, (1 - half) * 32 + 32)
                nc.vector.tensor_mul(
                    w[:, :, d_out],
                    t4[:, :, d_in],
                    tab_sb[:, nt, tcol + 64 + half * 32 : tcol + 96 + half * 32]
                    .unsqueeze(1)
                    .to_broadcast((128, 4, 32)),
                )
            nc.vector.tensor_add(dst[:], u[:], w[:])

        # ---- phase B: qkv + stats + rope + transposes, per 4-nt chunk ----
        for chunk in range(NT // CHUNK):
            nts = range(chunk * CHUNK, (chunk + 1) * CHUNK)
            for nt in nts:
                qk_ps = ps.tile([128, 512], F32, tag="s", name="qk_ps")
                v_ps = ps.tile([128, 512], F32, tag="s", name="v_ps")
                for dst, j0, j1 in ((qk_ps, 0, 512), (v_ps, 512, W3)):
                    jw = j1 - j0
                    for ct in range(CT):
                        nc.tensor.matmul(
                            dst[:, 0:jw],
                            lhsT=xT_sb[:, ct, nt * 128 : (nt + 1) * 128],
                            rhs=wq_sb[:, ct, j0:j1],
                            start=(ct == 0),
                            stop=(ct == CT - 1) and not with_qkv_bias,
                        )
                    if with_qkv_bias:
                        nc.tensor.matmul(
                            dst[:, 0:jw],
                            lhsT=ones_sb[:],
                            rhs=b_sb[:, j0:j1],
                            start=False,
                            stop=True,
                        )
                # PSUM evacuation on ACT (idle in this phase)
                nc.scalar.activation(
                    t_all[:, nt],
                    qk_ps[:].rearrange("p (h d) -> p h d", h=8),
                    COPY,
                )
                nc.scalar.activation(
                    v_sb[:, nt, :, 0:64],
                    v_ps[:, 0:256].rearrange("p (h d) -> p h d", h=NHC),
                    COPY,
                )
                # k-side rope first (feeds the DMA transposes), stats on GPSIMD
                kk = stage.tile([128, 4, HD], BF16, tag="kk", bufs=3, name="kk")
                rope4(kk, t_all[:, nt, 4:8], 128, nt)
                if with_ln_bias:
                    nc.vector.tensor_add(
                        kk[:], kk[:],
                        tln_sb[:, nt, 256:512].rearrange("p (h d) -> p h d", h=4),
                    )
                sq = stage.tile([128, 8, HD], BF16, name="sq")
                nc.gpsimd.tensor_mul(sq[:], t_all[:, nt], t_all[:, nt])
                nc.vector.tensor_reduce(
                    ssq_all[:, nt], sq[:], axis=mybir.AxisListType.X,
                    op=mybir.AluOpType.add,
                )
                flatk = kk.rearrange("p h d -> p (h d)")
                nc.sync.dma_start_transpose(
                    kT[0][:, nt * 128 : (nt + 1) * 128], flatk[:, 0:128]
                )
                nc.scalar.dma_start_transpose(
                    kT[1][:, nt * 128 : (nt + 1) * 128], flatk[:, 128:256]
                )

            # rsqrt for this chunk on DVE: rs = 1/sqrt(ssq/HD + eps)
            FL = CHUNK * 8
            dd = d_t[:, nts.start : nts.stop].rearrange("p a b -> p (a b)")
            nc.vector.tensor_scalar(
                dd,
                ssq_all[:, nts.start : nts.stop].rearrange("p a b -> p (a b)"),
                1.0 / HD, EPS, mybir.AluOpType.mult, mybir.AluOpType.add,
            )
            fi = small.tile([128, FL], F32, tag="rsq_fi", name="rsq_fi")
            nc.vector.tensor_copy(fi[:], dd.bitcast(I32))  # int32 -> f32
            nc.vector.tensor_scalar(
                fi[:], fi[:], -0.5, RSQRT_MAGIC,
                mybir.AluOpType.mult, mybir.AluOpType.add,
            )
            yi = small.tile([128, FL], I32, tag="rsq_yi", name="rsq_yi")
            nc.vector.tensor_copy(yi[:], fi[:])  # f32 -> int32
            y = yi[:].bitcast(F32)
            h_t = small.tile([128, FL], F32, tag="rsq_h", name="rsq_h")
            for _ in range(3):
                nc.vector.tensor_mul(h_t[:], y, y)
                nc.vector.tensor_mul(h_t[:], h_t[:], dd)
                nc.vector.tensor_scalar(
                    h_t[:], h_t[:], -0.5, 1.5,
                    mybir.AluOpType.mult, mybir.AluOpType.add,
                )
                nc.vector.tensor_mul(y, y, h_t[:])
            rsl = rs_sb[:, nts.start : nts.stop].rearrange("p a b -> p (a b)")
            nc.vector.tensor_copy(rsl, y)
            nc.vector.tensor_scalar(
                a_sb[:, nts.start : nts.stop].rearrange("p a b -> p (a b)"),
                rsl, EXPA, 0.0, mybir.AluOpType.mult, mybir.AluOpType.add,
            )

            # q-side rope + LN scale + transposes for this chunk
            for nt in nts:
                qq = stage.tile([128, 4, HD], BF16, tag="qq", bufs=3, name="qq")
                rope4(qq, t_all[:, nt, 0:4], 0, nt)
                if with_ln_bias:
                    nc.vector.tensor_add(
                        qq[:], qq[:],
                        tln_sb[:, nt, 0:256].rearrange("p (h d) -> p h d", h=4),
                    )
                nc.vector.tensor_mul(
                    qq[:],
                    qq[:],
                    rs_sb[:, nt, 0:4].unsqueeze(2).to_broadcast((128, 4, HD)),
                )
                flatq = qq.rearrange("p h d -> p (h d)")
                nc.sync.dma_start_transpose(
                    qT[0][:, nt * 128 : (nt + 1) * 128], flatq[:, 0:128]
                )
                nc.sync.dma_start_transpose(
                    qT[1][:, nt * 128 : (nt + 1) * 128], flatq[:, 128:256]
                )

        # ---- phase C: attention; normalization lags one block so the PE
        # always has the next block's S matmuls queued ----
        def emit_recip(oths):
            """ACT half of the normalization: rec = Exp(-Ln(den))."""
            lnd = small.tile([65, QBW], F32, tag="lnd", bufs=2, name="lnd")
            rrow = small.tile([65, QBW], F32, tag="rrow", bufs=2, name="rrow")
            for half in range(2):
                nc.scalar.activation(
                    lnd[64:65, half * 512 : (half + 1) * 512],
                    oths[half][64:65, :], LN_F,
                )
            nc.scalar.activation(rrow[64:65, :], lnd[64:65, :], EXP, scale=-1.0)
            return rrow

        def emit_normalize(qb, h, oths, rrow):
            """PE/DVE half: replicate rec across partitions, apply to o^T."""
            for half in range(2):
                rep_ps = ps.tile([128, 512], F32, tag="s", name="rep")
                nc.tensor.matmul(
                    rep_ps[:],
                    lhsT=onesf_sb[64:65, :],
                    rhs=rrow[64:65, half * 512 : (half + 1) * 512],
                    start=True,
                    stop=True,
                )
                rbc = stage.tile([64, 512], F32, tag="rbc", bufs=2, name="rbc")
                nc.vector.tensor_copy(rbc[:], rep_ps[0:64, :])
                nc.vector.tensor_mul(
                    oT[h][:, qb * QBW + half * 512 : qb * QBW + (half + 1) * 512],
                    oths[half][0:64, :],
                    rbc[:],
                )

        def emit_outproj(qb):
            for nt in range(qb * (NT // QB), (qb + 1) * (NT // QB)):
                op = ps.tile([128, 512], F32, tag="s", name="op")
                for h in range(NHC):
                    nc.tensor.matmul(
                        op[:],
                        lhsT=oT[h][:, nt * 128 : (nt + 1) * 128],
                        rhs=wo_sb[:, h, :],
                        start=(h == 0),
                        stop=(h == NHC - 1),
                    )
                ot = stage.tile([128, DIM], F32, tag="ot", bufs=2, name="ot")
                nc.vector.tensor_copy(ot[:], op[:])
                nc.sync.dma_start(out_d[nt * 128 : (nt + 1) * 128, :], ot[:])

        def emit_pv_step(blk, kt, half):
            qb, h, oths, etiles = blk
            nc.tensor.matmul(
                oths[half][0:65, :],
                lhsT=v_sb[:, kt, h, :],
                rhs=etiles[2 * kt + half][:],
                start=(kt == 0),
                stop=(kt == KT - 1),
            )

        # software pipeline over blocks: block i's S/exp stream carries block
        # i-1's PV matmuls interleaved (so the PE always has ready work and
        # the HAM clock stays warm) and block i-2's normalization.
        blocks = [(qb, h) for qb in range(QB) for h in range(NHC)]
        pv_blk = None      # block whose PV interleaves with the current S
        rc_blk = None      # block awaiting recip (its PV is done)
        rrow_p = None
        for qb, h in blocks:
            pair, hh = h // 2, h % 2
            dsl = slice(hh * 64, hh * 64 + 64)
            etiles = []
            oths = [
                ps.tile([128, 512], F32, tag="o", bufs=2, name=f"oT{half}")
                for half in range(2)
            ]
            for kt in range(KT):
                for half in range(2):
                    s_ps = ps.tile([128, 512], F32, tag="s", name="s")
                    nc.tensor.matmul(
                        s_ps[:],
                        lhsT=kT[pair][dsl, kt * 128 : (kt + 1) * 128],
                        rhs=qT[pair][
                            dsl,
                            qb * QBW + half * 512 : qb * QBW + (half + 1) * 512,
                        ],
                        start=True,
                        stop=True,
                    )
                    if kt in DVE_KT:
                        tf = fes.tile([128, 512], F32, tag="fe", name="fe")
                        nc.vector.tensor_scalar(
                            tf[:], s_ps[:], a_sb[:, kt, 4 + h : 5 + h], EXPB,
                            mybir.AluOpType.mult, mybir.AluOpType.add,
                        )
                        ei = exps.tile([128, 512], I16, tag="expS", name="expI")
                        nc.vector.tensor_copy(ei[:], tf[:])
                        etiles.append(ei[:].bitcast(BF16))
                    else:
                        e_sb = exps.tile([128, 512], BF16, tag="expS", name="expS")
                        nc.scalar.activation(
                            e_sb[:], s_ps[:], EXP,
                            scale=rs_sb[:, kt, 4 + h : 5 + h],
                        )
                        etiles.append(e_sb[:])
                    if pv_blk is not None:
                        emit_pv_step(pv_blk, kt, half)
                if kt == 1 and rc_blk is not None:
                    rrow_p = emit_recip(rc_blk[2])
                if kt == 6 and rc_blk is not None:
                    emit_normalize(rc_blk[0], rc_blk[1], rc_blk[2], rrow_p)
                    if rc_blk[1] == NHC - 1:
                        emit_outproj(rc_blk[0])
                    rc_blk = None
            rc_blk, pv_blk = pv_blk, (qb, h, oths, etiles)
        # drain the pipeline: last block's PV, last two normalizations
        if rc_blk is not None:
            emit_normalize(rc_blk[0], rc_blk[1], rc_blk[2], emit_recip(rc_blk[2]))
        for kt in range(KT):
            for half in range(2):
                emit_pv_step(pv_blk, kt, half)
        emit_normalize(pv_blk[0], pv_blk[1], pv_blk[2], emit_recip(pv_blk[2]))
        emit_outproj(QB - 1)

    return nc


# ---------------------------------------------------------------------------
# host-side input prep
# ---------------------------------------------------------------------------

def _prep_core_inputs(c, x, Wqkv_w, Wqkv_b, qn_g, qn_b, kn_g, kn_b, out_w):
    bf16 = ml_dtypes.bfloat16
    b, hg = c // 2, c % 2
    heads = np.arange(4 * hg, 4 * hg + 4)
    perm = np.concatenate([np.arange(0, HD, 2), np.arange(1, HD, 2)])

    # fold the per-head LN mean subtraction into the q/k weights
    Wq = Wqkv_w[0 * DIM : 1 * DIM].reshape(NH, HD, DIM)[heads]
    Wk = Wqkv_w[1 * DIM : 2 * DIM].reshape(NH, HD, DIM)[heads]
    Wq = (Wq - Wq.mean(axis=1, keepdims=True))[:, perm, :]
    Wk = (Wk - Wk.mean(axis=1, keepdims=True))[:, perm, :]
    Wv = Wqkv_w[2 * DIM : 3 * DIM].reshape(NH, HD, DIM)[heads]
    WT = np.concatenate(
        [
            Wq.reshape(256, DIM).T,
            Wk.reshape(256, DIM).T,
            Wv.reshape(256, DIM).T,
        ],
        axis=1,
    )  # [512, 768]
    wqkvT = np.ascontiguousarray(
        WT.reshape(CT, 128, W3).transpose(1, 0, 2).reshape(128, CT * W3)
    ).astype(bf16)

    # x transposed to [c, n] and tiled [128, CT, N]
    xTn = x[b].T  # [512, 2048]
    xT = np.ascontiguousarray(
        xTn.reshape(CT, 128, N).transpose(1, 0, 2).reshape(128, CT * N)
    ).astype(bf16)

    inv = 1.0 / (THETA ** (np.arange(0, HD, 2, dtype=np.float64) / HD))
    ang = np.arange(N, dtype=np.float64)[:, None] * inv[None, :]
    cos = np.cos(ang)
    sin = np.sin(ang)
    C2 = np.concatenate([cos, cos], axis=1)
    S2 = np.concatenate([-sin, sin], axis=1)
    SH = lambda v: np.concatenate([v[HD // 2 :], v[: HD // 2]])
    sc = HD ** -0.5
    g_q, g_k = qn_g[perm], kn_g[perm]
    C2q = C2 * g_q[None, :] * sc
    S2q = S2 * SH(g_q)[None, :] * sc
    C2k = C2 * g_k[None, :]
    S2k = S2 * SH(g_k)[None, :]
    tabN = np.concatenate([C2q, S2q, C2k, S2k], axis=1)  # [N, 256]
    tab = np.ascontiguousarray(
        tabN.reshape(NT, 128, 256).transpose(1, 0, 2).reshape(128, NT * 256)
    ).astype(bf16)

    # per-head Wo^T blocks [64, 512], stacked along free: [64, NHC*512]
    Wo = out_w.reshape(DIM, NH, HD)[:, heads, :]  # [512, 4, 64]
    woT = np.ascontiguousarray(
        Wo.transpose(1, 2, 0).reshape(NHC, HD, DIM).transpose(1, 0, 2).reshape(HD, NHC * DIM)
    ).astype(bf16)

    m = {"xT": xT, "wqkvT": wqkvT, "woT": woT, "tab": tab}

    if np.any(Wqkv_b != 0):
        bq = Wqkv_b[0 * DIM : 1 * DIM].reshape(NH, HD)[heads]
        bk = Wqkv_b[1 * DIM : 2 * DIM].reshape(NH, HD)[heads]
        bq = (bq - bq.mean(axis=1, keepdims=True))[:, perm]
        bk = (bk - bk.mean(axis=1, keepdims=True))[:, perm]
        bv = Wqkv_b[2 * DIM : 3 * DIM].reshape(NH, HD)[heads]
        brow = np.concatenate([bq.ravel(), bk.ravel(), bv.ravel()])[None, :]
        m["brow"] = brow.astype(bf16)
    if np.any(qn_b != 0) or np.any(kn_b != 0):
        b_q, b_k = qn_b[perm], kn_b[perm]
        Tq = (C2 * b_q[None, :] + S2 * SH(b_q)[None, :]) * sc
        Tk = C2 * b_k[None, :] + S2 * SH(b_k)[None, :]
        tlnN = np.concatenate([np.tile(Tq, (1, 4)), np.tile(Tk, (1, 4))], axis=1)
        m["tln"] = np.ascontiguousarray(
            tlnN.reshape(NT, 128, 512).transpose(1, 0, 2).reshape(128, NT * 512)
        ).astype(bf16)
    return m


_PROGRAM_CACHE = {}


def _get_program(with_qkv_bias, with_ln_bias, legalize=True):
    key = (with_qkv_bias, with_ln_bias, legalize)
    if key not in _PROGRAM_CACHE:
        nc = build_program(with_qkv_bias, with_ln_bias)
        if legalize:
            legalize_sync_waits(nc, 1)
        _PROGRAM_CACHE[key] = nc
    return _PROGRAM_CACHE[key]


def _run(inputs, trace=False):
    x = np.asarray(inputs["x"], np.float32)
    Wqkv_w = np.asarray(inputs["Wqkv_w"], np.float32)
    Wqkv_b = np.asarray(inputs["Wqkv_b"], np.float32)
    qn_g = np.asarray(inputs["qn_g"], np.float32)
    qn_b = np.asarray(inputs["qn_b"], np.float32)
    kn_g = np.asarray(inputs["kn_g"], np.float32)
    kn_b = np.asarray(inputs["kn_b"], np.float32)
    out_w = np.asarray(inputs["out_w"], np.float32)
    out_b = np.asarray(inputs["out_b"], np.float32)

    import time as _time

    _t = _time.time()
    in_maps = [
        _prep_core_inputs(c, x, Wqkv_w, Wqkv_b, qn_g, qn_b, kn_g, kn_b, out_w)
        for c in range(8)
    ]
    print(f"[kernel] host prep {_time.time()-_t:.1f}s", flush=True)
    _t = _time.time()
    nc = _get_program("brow" in in_maps[0], "tln" in in_maps[0])
    print(f"[kernel] program {_time.time()-_t:.1f}s", flush=True)
    _t = _time.time()
    res = run_bass_kernel_spmd(nc, in_maps, list(range(8)), trace=trace)
    print(f"[kernel] run {_time.time()-_t:.1f}s", flush=True)

    B = x.shape[0]
    bv = Wqkv_b[2 * DIM : 3 * DIM]
    out_bias = out_b + out_w @ bv
    out = np.empty((B, N, DIM), np.float32)
    for b in range(B):
        out[b] = res.results[2 * b]["outp"] + res.results[2 * b + 1]["outp"] + out_bias
    return out, res


def kernel(**inputs):
    out, _ = _run(inputs, trace=False)
    return out


# revision 22
# speedup vs baseline: 1.1910x; 1.1910x over previous
"""Trainium2 Bass kernel for nn_Attention_11836929868370.

8-core sharding: core c -> batch b = c//2, head group hg = c%2 (4 of 8 heads).
Each core computes its 4 heads' attention and a partial output projection;
the host sums the two partials per batch and adds the output bias.

Per-core pipeline (matmuls bf16, accumulation fp32 in PSUM):
  B.  Interleaved per 4-nt chunk:
        qkv = xT.T @ WqkvT (per-head LN mean subtraction folded into the
        weights host-side: W' = W - colmean(W)); ACT evacuates t and v from
        PSUM; GPSIMD computes ssq (sq + reduce); DVE ropes k (no LN scale -
        deferred to exp) and streams k/q transposes out on the sync/scalar
        DMA queues; after each chunk a small Newton rsqrt gives rs for the
        chunk and DVE ropes+scales q. Phase C can start once the first half
        of the q tiles exist.
  C.  per (head, 1024-wide q-block): S^T[nk,nq] = k'' @ q''.T on PE in
      [128,512] single-PSUM-bank tiles (6-deep ring so the PE runs ahead and
      LDWEIGHTS hide in the reorder window); softmax exp split between ACT
      (native Exp, per-partition scale = rs_k) and DVE (Schraudolph bit
      trick: f32 mad + f32->int16 convert writes the bf16 bit pattern of exp
      directly). PV with stationary [v_h | ones] gives o^T and the softmax
      denominator in row 64. Normalization lags one block: 1/den = Exp(-Ln)
      on ACT (same table as Exp, no reload), PE replicates it across
      partitions only after the next block's S matmuls are queued, DVE
      applies it.
  D.  out = oT.T @ WoT partial projection, staged to SBUF, DMA out. Host
      adds out_b (+ the v-bias contribution, which commutes through softmax
      averaging).
"""

import sys

if "/opt/trn_rl_repo" not in sys.path:
    sys.path.insert(0, "/opt/trn_rl_repo")

from contextlib import ExitStack

import ml_dtypes
import numpy as np

import concourse.bass as bass
import concourse.mybir as mybir
import concourse.tile as tile
from concourse.bass_utils import run_bass_kernel_spmd

BF16 = mybir.dt.bfloat16
F32 = mybir.dt.float32
I16 = mybir.dt.int16
I32 = mybir.dt.int32

DIM, NH, HD = 512, 8, 64
N = 2048
EPS = 1e-6
THETA = 10000.0
NT = N // 128          # 16 n-tiles
CT = DIM // 128        # 4 c-tiles
NHC = 4                # heads per core
QB = 2                 # q blocks of 1024
KT = NT                # key tiles
QBW = N // QB          # 1024
W3 = 768               # qkv weight columns per core (q 256 | k 256 | v 256)
CHUNK = 4              # nt per rsqrt chunk
RSQRT_MAGIC = float(0x5F3759DF)
# Schraudolph fast-exp constants for bf16 bit patterns via int16:
# bits(e^v) ~= round(v * 128/ln2 + (127*128 - 0.04367*128)); +0.5 assumes the
# f32->int16 convert truncates (if it rounds, the extra half-LSB is harmless).
EXPA = 184.66502308201368
EXPB = 16250.91
# which key-tiles' exp runs on DVE (rest on ACT)
DVE_KT = frozenset((2, 7, 12))


# ---------------------------------------------------------------------------
# sync-wait legalization: this walrus build rejects >1 sync wait per
# instruction ("Too many sync wait commands"), while Tile's sem assignment
# emits several. Excess waits are hoisted onto NoOps placed immediately
# before the instruction on the same engine, which preserves ordering.
# ---------------------------------------------------------------------------

def legalize_sync_waits(nc, max_waits=1):
    n = 0
    for fn in nc.m.functions:
        for bb in fn.blocks:
            new_insts = []
            for inst in bb.instructions:
                si = inst.sync_info
                if si is not None and si.on_wait and len(si.on_wait) > max_waits:
                    movable = [w for w in si.on_wait if w.wait_reg is None]
                    pinned = [w for w in si.on_wait if w.wait_reg is not None]
                    budget = max(max_waits - len(pinned), 0)
                    cut = len(movable) - budget
                    keep, excess = movable[cut:], movable[:cut]
                    for i in range(0, len(excess), max_waits):
                        nop = mybir.InstNoOp(
                            name=f"I-waitsplit-{n}",
                            engine=inst.engine,
                            text_hint="waitsplit",
                            sync_info=mybir.SyncInfo(
                                on_wait=excess[i : i + max_waits], on_update=[]
                            ),
                        )
                        n += 1
                        new_insts.append(nop)
                    si.on_wait = keep + pinned
                new_insts.append(inst)
            bb.instructions[:] = new_insts
    return n


# ---------------------------------------------------------------------------
# device program
# ---------------------------------------------------------------------------

def build_program(with_qkv_bias=False, with_ln_bias=False):
    nc = bass.Bass("TRN2", target_bir_lowering=False, debug=False, num_devices=8)

    # [128, CT, 2048]: x transposed (c on partitions) and cast to bf16
    xT_d = nc.dram_tensor("xT", [128, CT * N], BF16, kind="ExternalInput").ap()
    # [128, CT, 768]: wq(256 perm, mean-folded) | wk(256 perm, mean-folded) | wv(256)
    wq_d = nc.dram_tensor("wqkvT", [128, CT * W3], BF16, kind="ExternalInput").ap()
    wo_d = nc.dram_tensor("woT", [64, NHC * DIM], BF16, kind="ExternalInput").ap()
    # [128, NT, 256]: C2q | S2q | C2k | S2k  (gains, q-scale folded in)
    tab_d = nc.dram_tensor("tab", [128, NT * 256], BF16, kind="ExternalInput").ap()
    if with_qkv_bias:
        b_d = nc.dram_tensor("brow", [1, W3], BF16, kind="ExternalInput").ap()
    if with_ln_bias:
        tln_d = nc.dram_tensor("tln", [128, NT * 512], BF16, kind="ExternalInput").ap()
    out_d = nc.dram_tensor("outp", [N, DIM], F32, kind="ExternalOutput").ap()

    with tile.TileContext(nc) as tc, ExitStack() as ctx:
        consts = ctx.enter_context(tc.tile_pool(name="consts", bufs=1))
        pers = ctx.enter_context(tc.tile_pool(name="pers", bufs=1))
        stage = ctx.enter_context(tc.tile_pool(name="stage", bufs=4))
        small = ctx.enter_context(tc.tile_pool(name="small", bufs=4))
        exps = ctx.enter_context(tc.tile_pool(name="exps", bufs=52))
        fes = ctx.enter_context(tc.tile_pool(name="fes", bufs=3))
        ps = ctx.enter_context(tc.tile_pool(name="ps", bufs=6, space="PSUM"))

        # constants
        wq_sb = consts.tile([128, CT, W3], BF16)
        nc.sync.dma_start(wq_sb[:], wq_d.rearrange("p (t f) -> p t f", t=CT))
        xT_sb = consts.tile([128, CT, N], BF16)
        xT_dv = xT_d.rearrange("p (t f) -> p t f", t=CT)
        for ct in range(CT):
            nc.sync.dma_start(xT_sb[:, ct], xT_dv[:, ct])
        wo_sb = consts.tile([64, NHC, DIM], BF16)
        nc.sync.dma_start(wo_sb[:], wo_d.rearrange("p (t f) -> p t f", t=NHC))
        tab_sb = consts.tile([128, NT, 256], BF16)
        nc.sync.dma_start(tab_sb[:], tab_d.rearrange("p (t f) -> p t f", t=NT))
        # row 64 of this tile is the lhsT for the replicate matmul
        # (it must share its base partition with the PSUM denominator row)
        onesf_sb = consts.tile([65, 128], F32)
        nc.vector.memset(onesf_sb[:], 1.0)
        if with_qkv_bias:
            b_sb = consts.tile([1, W3], BF16)
            nc.sync.dma_start(b_sb[:], b_d)
            ones_sb = consts.tile([1, 128], BF16)
            nc.vector.memset(ones_sb[:], 1.0)
        if with_ln_bias:
            tln_sb = consts.tile([128, NT, 512], BF16)
            nc.sync.dma_start(tln_sb[:], tln_d.rearrange("p (t f) -> p t f", t=NT))

        # persistent intermediates
        qT = [pers.tile([128, N], BF16, name=f"qT{i}") for i in range(2)]
        kT = [pers.tile([128, N], BF16, name=f"kT{i}") for i in range(2)]
        oT = [pers.tile([64, N], BF16, name=f"oTh{i}") for i in range(NHC)]
        # v with a ones column per head: PV row 64 is the softmax denominator
        v_sb = pers.tile([128, KT, NHC, 65], BF16)
        t_all = pers.tile([128, NT, 8, HD], BF16)
        ssq_all = pers.tile([128, NT, 8], F32)
        rs_sb = pers.tile([128, NT, 8], F32)
        a_sb = pers.tile([128, NT, 8], F32)
        d_t = pers.tile([128, NT, 8], F32, name="rsq_d")

        nc.vector.memset(v_sb[:, :, :, 64], 1.0)

        COPY = mybir.ActivationFunctionType.Copy
        EXP = mybir.ActivationFunctionType.Exp
        LN_F = mybir.ActivationFunctionType.Ln

        def rope4(dst, t4, tcol, nt):
            """dst = rope of t4 (4 heads) using tab columns at tcol."""
            u = stage.tile([128, 4, HD], BF16, name="u")
            w = stage.tile([128, 4, HD], BF16, name="w")
            nc.vector.tensor_mul(
                u[:],
                t4,
                tab_sb[:, nt, tcol : tcol + 64].unsqueeze(1).to_broadcast((128, 4, HD)),
            )
            for half in (0, 1):
                d_out = slice(half * 32, half * 32 + 32)
                d_in = slice((1 - half) * 32, (1 - half) * 32 + 32)
                nc.vector.tensor_mul(
                    w[:, :, d_out],
                    t4[:, :, d_in],
                    tab_sb[:, nt, tcol + 64 + half * 32 : tcol + 96 + half * 32]
                    .unsqueeze(1)
                    .to_broadcast((128, 4, 32)),
                )
            nc.vector.tensor_add(dst[:], u[:], w[:])

        # ---- phase B: qkv + stats + rope + transposes, per 4-nt chunk ----
        for chunk in range(NT // CHUNK):
            nts = range(chunk * CHUNK, (chunk + 1) * CHUNK)
            for nt in nts:
                qk_ps = ps.tile([128, 512], F32, tag="s", name="qk_ps")
                v_ps = ps.tile([128, 512], F32, tag="s", name="v_ps")
                for dst, j0, j1 in ((qk_ps, 0, 512), (v_ps, 512, W3)):
                    jw = j1 - j0
                    for ct in range(CT):
                        nc.tensor.matmul(
                            dst[:, 0:jw],
                            lhsT=xT_sb[:, ct, nt * 128 : (nt + 1) * 128],
                            rhs=wq_sb[:, ct, j0:j1],
                            start=(ct == 0),
                            stop=(ct == CT - 1) and not with_qkv_bias,
                        )
                    if with_qkv_bias:
                        nc.tensor.matmul(
                            dst[:, 0:jw],
                            lhsT=ones_sb[:],
                            rhs=b_sb[:, j0:j1],
                            start=False,
                            stop=True,
                        )
                # PSUM evacuation on ACT (idle in this phase)
                nc.scalar.activation(
                    t_all[:, nt],
                    qk_ps[:].rearrange("p (h d) -> p h d", h=8),
                    COPY,
                )
                nc.scalar.activation(
                    v_sb[:, nt, :, 0:64],
                    v_ps[:, 0:256].rearrange("p (h d) -> p h d", h=NHC),
                    COPY,
                )
                # k-side rope first (feeds the DMA transposes), stats on GPSIMD
                kk = stage.tile([128, 4, HD], BF16, tag="kk", bufs=3, name="kk")
                rope4(kk, t_all[:, nt, 4:8], 128, nt)
                if with_ln_bias:
                    nc.vector.tensor_add(
                        kk[:], kk[:],
                        tln_sb[:, nt, 256:512].rearrange("p (h d) -> p h d", h=4),
                    )
                sq = stage.tile([128, 8, HD], BF16, name="sq")
                nc.gpsimd.tensor_mul(sq[:], t_all[:, nt], t_all[:, nt])
                nc.vector.tensor_reduce(
                    ssq_all[:, nt], sq[:], axis=mybir.AxisListType.X,
                    op=mybir.AluOpType.add,
                )
                flatk = kk.rearrange("p h d -> p (h d)")
                nc.sync.dma_start_transpose(
                    kT[0][:, nt * 128 : (nt + 1) * 128], flatk[:, 0:128]
                )
                nc.scalar.dma_start_transpose(
                    kT[1][:, nt * 128 : (nt + 1) * 128], flatk[:, 128:256]
                )

            # rsqrt for this chunk on DVE: rs = 1/sqrt(ssq/HD + eps)
            FL = CHUNK * 8
            dd = d_t[:, nts.start : nts.stop].rearrange("p a b -> p (a b)")
            nc.vector.tensor_scalar(
                dd,
                ssq_all[:, nts.start : nts.stop].rearrange("p a b -> p (a b)"),
                1.0 / HD, EPS, mybir.AluOpType.mult, mybir.AluOpType.add,
            )
            fi = small.tile([128, FL], F32, tag="rsq_fi", name="rsq_fi")
            nc.vector.tensor_copy(fi[:], dd.bitcast(I32))  # int32 -> f32
            nc.vector.tensor_scalar(
                fi[:], fi[:], -0.5, RSQRT_MAGIC,
                mybir.AluOpType.mult, mybir.AluOpType.add,
            )
            yi = small.tile([128, FL], I32, tag="rsq_yi", name="rsq_yi")
            nc.vector.tensor_copy(yi[:], fi[:])  # f32 -> int32
            y = yi[:].bitcast(F32)
            h_t = small.tile([128, FL], F32, tag="rsq_h", name="rsq_h")
            for _ in range(3):
                nc.vector.tensor_mul(h_t[:], y, y)
                nc.vector.tensor_mul(h_t[:], h_t[:], dd)
                nc.vector.tensor_scalar(
                    h_t[:], h_t[:], -0.5, 1.5,
                    mybir.AluOpType.mult, mybir.AluOpType.add,
                )
                nc.vector.tensor_mul(y, y, h_t[:])
            rsl = rs_sb[:, nts.start : nts.stop].rearrange("p a b -> p (a b)")
            nc.vector.tensor_copy(rsl, y)
            nc.vector.tensor_scalar(
                a_sb[:, nts.start : nts.stop].rearrange("p a b -> p (a b)"),
                rsl, EXPA, 0.0, mybir.AluOpType.mult, mybir.AluOpType.add,
            )

            # q-side rope + LN scale + transposes for this chunk
            for nt in nts:
                qq = stage.tile([128, 4, HD], BF16, tag="qq", bufs=3, name="qq")
                rope4(qq, t_all[:, nt, 0:4], 0, nt)
                if with_ln_bias:
                    nc.vector.tensor_add(
                        qq[:], qq[:],
                        tln_sb[:, nt, 0:256].rearrange("p (h d) -> p h d", h=4),
                    )
                nc.vector.tensor_mul(
                    qq[:],
                    qq[:],
                    rs_sb[:, nt, 0:4].unsqueeze(2).to_broadcast((128, 4, HD)),
                )
                flatq = qq.rearrange("p h d -> p (h d)")
                nc.sync.dma_start_transpose(
                    qT[0][:, nt * 128 : (nt + 1) * 128], flatq[:, 0:128]
                )
                nc.scalar.dma_start_transpose(
                    qT[1][:, nt * 128 : (nt + 1) * 128], flatq[:, 128:256]
                )

        # ---- phase C: attention; normalization lags one block so the PE
        # always has the next block's S matmuls queued ----
        def emit_recip(oths):
            """ACT half of the normalization: rec = Exp(-Ln(den))."""
            lnd = small.tile([65, QBW], F32, tag="lnd", bufs=2, name="lnd")
            rrow = small.tile([65, QBW], F32, tag="rrow", bufs=2, name="rrow")
            for half in range(2):
                nc.scalar.activation(
                    lnd[64:65, half * 512 : (half + 1) * 512],
                    oths[half][64:65, :], LN_F,
                )
            nc.scalar.activation(rrow[64:65, :], lnd[64:65, :], EXP, scale=-1.0)
            return rrow

        def emit_normalize(qb, h, oths, rrow):
            """PE/DVE half: replicate rec across partitions, apply to o^T."""
            for half in range(2):
                rep_ps = ps.tile([128, 512], F32, tag="s", name="rep")
                nc.tensor.matmul(
                    rep_ps[:],
                    lhsT=onesf_sb[64:65, :],
                    rhs=rrow[64:65, half * 512 : (half + 1) * 512],
                    start=True,
                    stop=True,
                )
                rbc = stage.tile([64, 512], F32, tag="rbc", bufs=2, name="rbc")
                nc.vector.tensor_copy(rbc[:], rep_ps[0:64, :])
                nc.vector.tensor_mul(
                    oT[h][:, qb * QBW + half * 512 : qb * QBW + (half + 1) * 512],
                    oths[half][0:64, :],
                    rbc[:],
                )

        def emit_outproj(qb):
            for nt in range(qb * (NT // QB), (qb + 1) * (NT // QB)):
                op = ps.tile([128, 512], F32, tag="s", name="op")
                for h in range(NHC):
                    nc.tensor.matmul(
                        op[:],
                        lhsT=oT[h][:, nt * 128 : (nt + 1) * 128],
                        rhs=wo_sb[:, h, :],
                        start=(h == 0),
                        stop=(h == NHC - 1),
                    )
                ot = stage.tile([128, DIM], F32, tag="ot", bufs=2, name="ot")
                nc.vector.tensor_copy(ot[:], op[:])
                nc.sync.dma_start(out_d[nt * 128 : (nt + 1) * 128, :], ot[:])

        def emit_pv_step(blk, kt, half):
            qb, h, oths, etiles = blk
            nc.tensor.matmul(
                oths[half][0:65, :],
                lhsT=v_sb[:, kt, h, :],
                rhs=etiles[2 * kt + half][:],
                start=(kt == 0),
                stop=(kt == KT - 1),
            )

        # software pipeline over blocks: block i's S/exp stream carries block
        # i-1's PV matmuls interleaved (so the PE always has ready work and
        # the HAM clock stays warm) and block i-2's normalization.
        blocks = [(qb, h) for qb in range(QB) for h in range(NHC)]
        pv_blk = None      # block whose PV interleaves with the current S
        rc_blk = None      # block awaiting recip (its PV is done)
        rrow_p = None
        for qb, h in blocks:
            pair, hh = h // 2, h % 2
            dsl = slice(hh * 64, hh * 64 + 64)
            etiles = []
            oths = [
                ps.tile([128, 512], F32, tag="o", bufs=2, name=f"oT{half}")
                for half in range(2)
            ]
            for kt in range(KT):
                for half in range(2):
                    s_ps = ps.tile([128, 512], F32, tag="s", name="s")
                    nc.tensor.matmul(
                        s_ps[:],
                        lhsT=kT[pair][dsl, kt * 128 : (kt + 1) * 128],
                        rhs=qT[pair][
                            dsl,
                            qb * QBW + half * 512 : qb * QBW + (half + 1) * 512,
                        ],
                        start=True,
                        stop=True,
                    )
                    if kt in DVE_KT:
                        tf = fes.tile([128, 512], F32, tag="fe", name="fe")
                        nc.vector.tensor_scalar(
                            tf[:], s_ps[:], a_sb[:, kt, 4 + h : 5 + h], EXPB,
                            mybir.AluOpType.mult, mybir.AluOpType.add,
                        )
                        ei = exps.tile([128, 512], I16, tag="expS", name="expI")
                        nc.vector.tensor_copy(ei[:], tf[:])
                        etiles.append(ei[:].bitcast(BF16))
                    else:
                        e_sb = exps.tile([128, 512], BF16, tag="expS", name="expS")
                        nc.scalar.activation(
                            e_sb[:], s_ps[:], EXP,
                            scale=rs_sb[:, kt, 4 + h : 5 + h],
                        )
                        etiles.append(e_sb[:])
                    if pv_blk is not None:
                        emit_pv_step(pv_blk, kt, half)
                if kt == 1 and rc_blk is not None:
                    rrow_p = emit_recip(rc_blk[2])
                if kt == 6 and rc_blk is not None:
                    emit_normalize(rc_blk[0], rc_blk[1], rc_blk[2], rrow_p)
                    if rc_blk[1] == NHC - 1:
                        emit_outproj(rc_blk[0])
                    rc_blk = None
            rc_blk, pv_blk = pv_blk, (qb, h, oths, etiles)
        # drain the pipeline: last block's PV, last two normalizations
        if rc_blk is not None:
            emit_normalize(rc_blk[0], rc_blk[1], rc_blk[2], emit_recip(rc_blk[2]))
        for kt in range(KT):
            for half in range(2):
                emit_pv_step(pv_blk, kt, half)
        emit_normalize(pv_blk[0], pv_blk[1], pv_blk[2], emit_recip(pv_blk[2]))
        emit_outproj(QB - 1)

    return nc


# ---------------------------------------------------------------------------
# host-side input prep
# ---------------------------------------------------------------------------

def _prep_core_inputs(c, x, Wqkv_w, Wqkv_b, qn_g, qn_b, kn_g, kn_b, out_w):
    bf16 = ml_dtypes.bfloat16
    b, hg = c // 2, c % 2
    heads = np.arange(4 * hg, 4 * hg + 4)
    perm = np.concatenate([np.arange(0, HD, 2), np.arange(1, HD, 2)])

    # fold the per-head LN mean subtraction into the q/k weights
    Wq = Wqkv_w[0 * DIM : 1 * DIM].reshape(NH, HD, DIM)[heads]
    Wk = Wqkv_w[1 * DIM : 2 * DIM].reshape(NH, HD, DIM)[heads]
    Wq = (Wq - Wq.mean(axis=1, keepdims=True))[:, perm, :]
    Wk = (Wk - Wk.mean(axis=1, keepdims=True))[:, perm, :]
    Wv = Wqkv_w[2 * DIM : 3 * DIM].reshape(NH, HD, DIM)[heads]
    WT = np.concatenate(
        [
            Wq.reshape(256, DIM).T,
            Wk.reshape(256, DIM).T,
            Wv.reshape(256, DIM).T,
        ],
        axis=1,
    )  # [512, 768]
    wqkvT = np.ascontiguousarray(
        WT.reshape(CT, 128, W3).transpose(1, 0, 2).reshape(128, CT * W3)
    ).astype(bf16)

    # x transposed to [c, n] and tiled [128, CT, N]
    xTn = x[b].T  # [512, 2048]
    xT = np.ascontiguousarray(
        xTn.reshape(CT, 128, N).transpose(1, 0, 2).reshape(128, CT * N)
    ).astype(bf16)

    inv = 1.0 / (THETA ** (np.arange(0, HD, 2, dtype=np.float64) / HD))
    ang = np.arange(N, dtype=np.float64)[:, None] * inv[None, :]
    cos = np.cos(ang)
    sin = np.sin(ang)
    C2 = np.concatenate([cos, cos], axis=1)
    S2 = np.concatenate([-sin, sin], axis=1)
    SH = lambda v: np.concatenate([v[HD // 2 :], v[: HD // 2]])
    sc = HD ** -0.5
    g_q, g_k = qn_g[perm], kn_g[perm]
    C2q = C2 * g_q[None, :] * sc
    S2q = S2 * SH(g_q)[None, :] * sc
    C2k = C2 * g_k[None, :]
    S2k = S2 * SH(g_k)[None, :]
    tabN = np.concatenate([C2q, S2q, C2k, S2k], axis=1)  # [N, 256]
    tab = np.ascontiguousarray(
        tabN.reshape(NT, 128, 256).transpose(1, 0, 2).reshape(128, NT * 256)
    ).astype(bf16)

    # per-head Wo^T blocks [64, 512], stacked along free: [64, NHC*512]
    Wo = out_w.reshape(DIM, NH, HD)[:, heads, :]  # [512, 4, 64]
    woT = np.ascontiguousarray(
        Wo.transpose(1, 2, 0).reshape(NHC, HD, DIM).transpose(1, 0, 2).reshape(HD, NHC * DIM)
    ).astype(bf16)

    m = {"xT": xT, "wqkvT": wqkvT, "woT": woT, "tab": tab}

    if np.any(Wqkv_b != 0):
        bq = Wqkv_b[0 * DIM : 1 * DIM].reshape(NH, HD)[heads]
        bk = Wqkv_b[1 * DIM : 2 * DIM].reshape(NH, HD)[heads]
        bq = (bq - bq.mean(axis=1, keepdims=True))[:, perm]
        bk = (bk - bk.mean(axis=1, keepdims=True))[:, perm]
        bv = Wqkv_b[2 * DIM : 3 * DIM].reshape(NH, HD)[heads]
        brow = np.concatenate([bq.ravel(), bk.ravel(), bv.ravel()])[None, :]
        m["brow"] = brow.astype(bf16)
    if np.any(qn_b != 0) or np.any(kn_b != 0):
        b_q, b_k = qn_b[perm], kn_b[perm]
        Tq = (C2 * b_q[None, :] + S2 * SH(b_q)[None, :]) * sc
        Tk = C2 * b_k[None, :] + S2 * SH(b_k)[None, :]
        tlnN = np.concatenate([np.tile(Tq, (1, 4)), np.tile(Tk, (1, 4))], axis=1)
        m["tln"] = np.ascontiguousarray(
            tlnN.reshape(NT, 128, 512).transpose(1, 0, 2).reshape(128, NT * 512)
        ).astype(bf16)
    return m


_PROGRAM_CACHE = {}


def _get_program(with_qkv_bias, with_ln_bias, legalize=True):
    key = (with_qkv_bias, with_ln_bias, legalize)
    if key not in _PROGRAM_CACHE:
        nc = build_program(with_qkv_bias, with_ln_bias)
        if legalize:
            legalize_sync_waits(nc, 1)
        _PROGRAM_CACHE[key] = nc
    return _PROGRAM_CACHE[key]


def _run(inputs, trace=False):
    x = np.asarray(inputs["x"], np.float32)
    Wqkv_w = np.asarray(inputs["Wqkv_w"], np.float32)
    Wqkv_b = np.asarray(inputs["Wqkv_b"], np.float32)
    qn_g = np.asarray(inputs["qn_g"], np.float32)
    qn_b = np.asarray(inputs["qn_b"], np.float32)
    kn_g = np.asarray(inputs["kn_g"], np.float32)
    kn_b = np.asarray(inputs["kn_b"], np.float32)
    out_w = np.asarray(inputs["out_w"], np.float32)
    out_b = np.asarray(inputs["out_b"], np.float32)

    import time as _time

    _t = _time.time()
    in_maps = [
        _prep_core_inputs(c, x, Wqkv_w, Wqkv_b, qn_g, qn_b, kn_g, kn_b, out_w)
        for c in range(8)
    ]
    print(f"[kernel] host prep {_time.time()-_t:.1f}s", flush=True)
    _t = _time.time()
    nc = _get_program("brow" in in_maps[0], "tln" in in_maps[0])
    print(f"[kernel] program {_time.time()-_t:.1f}s", flush=True)
    _t = _time.time()
    res = run_bass_kernel_spmd(nc, in_maps, list(range(8)), trace=trace)
    print(f"[kernel] run {_time.time()-_t:.1f}s", flush=True)

    B = x.shape[0]
    bv = Wqkv_b[2 * DIM : 3 * DIM]
    out_bias = out_b + out_w @ bv
    out = np.empty((B, N, DIM), np.float32)
    for b in range(B):
        out[b] = res.results[2 * b]["outp"] + res.results[2 * b + 1]["outp"] + out_bias
    return out, res


def kernel(**inputs):
    out, _ = _run(inputs, trace=False)
    return out
